# revision 9
# baseline (speedup 1.0000x reference)
"""Averaged Hausdorff loss distributed Trainium2 kernel (8 NeuronCores).

reference:
    d[i,j] = ||set1_i - set2_j||  (sets are [8192, 128] f32)
    out = 0.5 * (sum_i min_j d + sum_j min_i d)

Strategy: shard set1 rows across the 8 cores (1024 rows each); every core
holds all of set2. Work with s[i,j] = 2*a_i.b_j - ||a_i||^2 - ||b_j||^2
= -d^2 so both reductions are maxes.

v2: one fused fp8-e4m3 DoubleRow matmul per psum region computes
    psum = 2a.b + (SHIFT - x2_i) + (-y2_j)
directly: features ride partitions 0-63 (2 per partition via DoubleRow),
and partitions 64-66 carry the bias as rank-1 terms, each split into
three fp8 values (h/m/l) so the bias is exact to ~0.04.

Drain (the bottleneck): per i-tile of [128, 8192] psum, split in 4
groups of 2048:
  - groups 0..2 -> ACT: E = exp(beta*psum - beta*SHIFT) in bf16 with
    accum_out = row-softmin partial sums (host does ln).
  - group 3 -> DVE tensor_tensor_reduce: slab = psum (bf16) plus exact
    row max for free; then colsacc = max(colsacc, slab).
  - col max over E (j < 6144) runs 2 chunks on DVE + 1 chunk on GPSIMD,
    deferred one tile so psum drains stay at the head of DVE's queue.
Host: ln/sqrt + merge of E-domain cols, raw slab cols, soft/exact rows.
"""

import os
import sys

sys.path.insert(0, "/opt/trn_rl_repo")

import ml_dtypes
import numpy as np

import concourse.bass as bass
import concourse.mybir as mybir
from concourse import bacc
from concourse.tile import TileContext

P = 128
N = 8192  # set1 rows (total)
M = 8192  # set2 rows
D = 128
NCORES = 8
NSH = N // NCORES  # 1024 rows per core
N_IT = NSH // P  # 8 i-tiles per core
G = 2048  # psum group width (4 banks); 2 groups ping-pong
N_G = M // G  # 4 groups per i-tile
MMW = int(os.environ.get("K_MMW", "512"))  # matmul moving width

BETA = 0.3
SHIFT = 155.0  # raw-slab bf16 centering shift
# V_G=1 (tensor_tensor_reduce psum drain on DVE) compiles but faults at
# runtime on this toolchain — default to the all-ACT drain, which is correct.
V_G = int(os.environ.get("K_V", "0"))  # DVE-drained groups per tile (last ones)
A_G = N_G - V_G  # ACT-drained groups per tile
EW = A_G * G  # E width (ACT-drained j range)
SW = V_G * G  # slab width (DVE-drained j range)
# gpsimd TENSOR_TENSOR is rejected by walrus codegen on trn2 (not a valid
# Pool-engine opcode) — keep colacc chunks on DVE.
GP_CHUNKS = int(os.environ.get("K_GP", "0"))  # colacc chunks on gpsimd
N_CH = EW // G  # colacc chunks (2048 each)

BF = mybir.dt.bfloat16
F32 = mybir.dt.float32
F8 = mybir.dt.float8e4
MAX = mybir.AluOpType.max
DR = mybir.MatmulPerfMode.DoubleRow


def build_nc():
    nc = bacc.Bacc("TRN2")

    adr = nc.declare_dram_parameter("adr", [P, 2, NSH], F8, isOutput=False)
    bdr = nc.declare_dram_parameter("bdr", [P, N_G, 2, G], F8, isOutput=False)
    colE = nc.declare_dram_parameter("colE", [P, EW], BF, isOutput=True)
    colS = nc.declare_dram_parameter("colS", [P, max(SW, 1)], BF, isOutput=True)
    rowmaxS = nc.declare_dram_parameter("rowmaxS", [P, max(N_IT * V_G, 1)], F32, isOutput=True)
    rowsumE = nc.declare_dram_parameter(
        "rowsumE", [P, N_IT * A_G + 1], F32, isOutput=True
    )

    with TileContext(nc) as tc:
        with (
            tc.tile_pool(name="const", bufs=1) as cpool,
            tc.tile_pool(name="s", bufs=3) as spool,
            tc.tile_pool(name="fold", bufs=2) as fpool,
            tc.tile_pool(name="psum", bufs=2, space="PSUM") as ppool,
        ):
            adr_sb = cpool.tile([P, 2, NSH], F8, tag="adr")
            bdr_sb = cpool.tile([P, N_G, 2, G], F8, tag="bdr")
            colacc = cpool.tile([P, EW], BF, tag="colacc")
            colsacc = cpool.tile([P, max(SW, 1)], BF, tag="colsacc")
            rmax_sb = cpool.tile([P, max(N_IT * V_G, 1)], F32, tag="rmax")
            rsum_sb = cpool.tile([P, N_IT * A_G + 1], F32, tag="rsum")
            warm = cpool.tile([P, 1], F32, tag="warm")
            nbias = cpool.tile([P, 1], F32, tag="nbias")
            nc.vector.memset(nbias[:], -BETA * SHIFT)
            neginf = cpool.tile([P, G], BF, tag="neginf")
            nc.vector.memset(neginf[:], -3.0e38)

            # input DMAs: the first matmul needs adr + bdr group 0; spread
            # across queues so dispatch overheads overlap. gpsimd dispatch
            # is cheapest; sync next. Avoid scalar (ACT is drain-critical).
            nc.gpsimd.dma_start(out=bdr_sb[:, 0, :, :], in_=bdr[:, 0, :, :])
            nc.sync.dma_start(out=adr_sb[:], in_=adr[:])
            nc.sync.dma_start(out=bdr_sb[:, 1, :, :], in_=bdr[:, 1, :, :])
            nc.gpsimd.dma_start(out=bdr_sb[:, 2, :, :], in_=bdr[:, 2, :, :])
            nc.sync.dma_start(out=bdr_sb[:, 3, :, :], in_=bdr[:, 3, :, :])

            # dummy Exp pulls the ACT_TABLE_LOAD off the first eviction
            nc.vector.memset(warm[:], 0.0)
            nc.scalar.activation(
                warm[:],
                warm[:],
                mybir.ActivationFunctionType.Exp,
                bias=0.0,
                scale=1.0,
            )

            def col_update(pit, pe, pprev):
                """Deferred colacc update for tile `pit` (runs one tile late
                so DVE/GP psum-adjacent work stays at the queue head)."""
                if pit == 0:
                    return  # e0 is consumed by tile 1's update
                for c in range(N_CH):
                    lo, hi = c * G, (c + 1) * G
                    eng = nc.gpsimd if c >= N_CH - GP_CHUNKS else nc.vector
                    if pit == 1:
                        eng.tensor_max(
                            colacc[:, lo:hi], pprev[:, lo:hi], pe[:, lo:hi]
                        )
                    else:
                        eng.tensor_max(
                            colacc[:, lo:hi], colacc[:, lo:hi], pe[:, lo:hi]
                        )
                    if pit == N_IT - 1:
                        nc.sync.dma_start(out=colE[:, lo:hi], in_=colacc[:, lo:hi])

            e_prev = None
            pending = None
            for it in range(N_IT):
                lhs = adr_sb[:, :, it * P : (it + 1) * P]
                e_full = spool.tile([P, EW], BF, tag="e")
                for g in range(N_G):
                    pg = ppool.tile([P, G], F32, tag="pg")
                    # the very first group is split in half so the eviction
                    # stream starts earlier (cold fills are slow)
                    subs = 2 if (it == 0 and g == 0) else 1
                    for s in range(subs):
                        W = G // subs
                        for jj in range(W // MMW):
                            jo = s * W + jj * MMW
                            nc.tensor.matmul(
                                pg[:, jo : jo + MMW],
                                lhs,
                                bdr_sb[:, g, :, jo : jo + MMW],
                                start=True,
                                stop=True,
                                perf_mode=DR,
                            )
                        if subs == 2:
                            nc.scalar.activation(
                                e_full[:, s * W : (s + 1) * W],
                                pg[:, s * W : (s + 1) * W],
                                mybir.ActivationFunctionType.Exp,
                                bias=nbias[:],
                                scale=BETA,
                                accum_out=rsum_sb[:, N_IT * A_G : N_IT * A_G + 1]
                                if s == 1
                                else rsum_sb[:, 0:1],
                            )
                    if subs == 2:
                        continue
                    if g >= A_G:
                        # DVE drains: slab = psum (bf16) + exact row max
                        sidx = g - A_G
                        slab = fpool.tile([P, G], BF, tag="slab")
                        nc.vector.tensor_tensor_reduce(
                            out=slab[:],
                            in0=pg[:],
                            in1=neginf[:],
                            scale=1.0,
                            scalar=-3.0e38,
                            op0=MAX,
                            op1=MAX,
                            accum_out=rmax_sb[:, it * V_G + sidx : it * V_G + sidx + 1],
                        )
                        srange = colsacc[:, sidx * G : (sidx + 1) * G]
                        if it == 0:
                            nc.vector.tensor_max(srange, slab[:], slab[:])
                        else:
                            nc.vector.tensor_max(srange, srange, slab[:])
                        if it == N_IT - 1:
                            nc.sync.dma_start(
                                out=colS[:, sidx * G : (sidx + 1) * G], in_=srange
                            )
                    else:
                        # ACT evicts: E = exp(beta*psum - beta*SHIFT) = e^{beta*s}
                        nc.scalar.activation(
                            e_full[:, g * G : (g + 1) * G],
                            pg[:],
                            mybir.ActivationFunctionType.Exp,
                            bias=nbias[:],
                            scale=BETA,
                            accum_out=rsum_sb[:, it * A_G + g : it * A_G + g + 1],
                        )

                if pending is not None:
                    col_update(pending[0], pending[1], pending[2])
                pending = (it, e_full, e_prev)
                e_prev = e_full
            col_update(pending[0], pending[1], pending[2])

            if V_G == 0:
                nc.vector.memset(colsacc[:], 0.0)
                nc.vector.memset(rmax_sb[:], 0.0)
                nc.sync.dma_start(out=colS.ap(), in_=colsacc[:])
            nc.sync.dma_start(out=rowmaxS.ap(), in_=rmax_sb[:])
            nc.sync.dma_start(out=rowsumE.ap(), in_=rsum_sb[:])

    nc.finalize()
    return nc


def _split_f8(v, n=3):
    """Decompose v (f64 vector) into n fp8-e4m3 parts summing to ~v."""
    parts = []
    r = np.asarray(v, dtype=np.float64).copy()
    for _ in range(n):
        p = r.astype(np.float32).astype(ml_dtypes.float8_e4m3)
        parts.append(p)
        r -= p.astype(np.float64)
    return parts


def make_in_maps(set1: np.ndarray, set2: np.ndarray):
    set1 = np.ascontiguousarray(set1, dtype=np.float32)
    set2 = np.ascontiguousarray(set2, dtype=np.float32)
    x2 = (set1.astype(np.float64) ** 2).sum(axis=1)  # [N] f64
    y2 = (set2.astype(np.float64) ** 2).sum(axis=1)  # [M] f64

    syh, sym, syl = _split_f8(-y2)

    # bdr: [128, N_G, 2, G] fp8. partitions 0-63: features (feat = q*64+p);
    # partitions 64-66: bias columns (ones against the x2 weights, -y2
    # splits against the ones weights).
    bt = set2.T  # [128, M] f32
    bdr = np.zeros((P, N_G, 2, G), dtype=ml_dtypes.float8_e4m3)
    btg = bt.reshape(P, N_G, G)
    bdr[0:64, :, 0, :] = btg[0:64].astype(ml_dtypes.float8_e4m3)
    bdr[0:64, :, 1, :] = btg[64:128].astype(ml_dtypes.float8_e4m3)
    one8 = np.float32(1.0).astype(ml_dtypes.float8_e4m3)
    for p in (64, 65, 66):
        bdr[p, :, 0, :] = one8
    bdr[64, :, 1, :] = syh.reshape(N_G, G)
    bdr[65, :, 1, :] = sym.reshape(N_G, G)
    bdr[66, :, 1, :] = syl.reshape(N_G, G)
    bdr = np.ascontiguousarray(bdr)

    in_maps = []
    for c in range(NCORES):
        rows = slice(c * NSH, (c + 1) * NSH)
        a2t = (2.0 * set1[rows]).T  # [128, NSH] f32
        sxh, sxm, sxl = _split_f8(SHIFT - x2[rows])
        adr = np.zeros((P, 2, NSH), dtype=ml_dtypes.float8_e4m3)
        adr[0:64, 0, :] = a2t[0:64].astype(ml_dtypes.float8_e4m3)
        adr[0:64, 1, :] = a2t[64:128].astype(ml_dtypes.float8_e4m3)
        adr[64, 0, :] = sxh
        adr[65, 0, :] = sxm
        adr[66, 0, :] = sxl
        for p in (64, 65, 66):
            adr[p, 1, :] = one8
        in_maps.append({"adr": np.ascontiguousarray(adr), "bdr": bdr})
    return in_maps


def combine(results) -> np.float32:
    # ---- term2: col mins ----
    colmaxE = np.zeros(EW, dtype=np.float64)
    colmaxS = np.full(SW, -np.inf)
    for r in results:
        ce = np.asarray(r["colE"]).astype(np.float32)  # [P, EW]
        np.maximum(colmaxE, ce.max(axis=0).astype(np.float64), out=colmaxE)
        cs = np.asarray(r["colS"]).astype(np.float32)  # [P, SW]
        np.maximum(colmaxS, cs.max(axis=0).astype(np.float64), out=colmaxS)
    d2colE = -np.log(np.maximum(colmaxE, 1e-300)) / BETA
    if V_G > 0:
        d2colS = SHIFT - colmaxS
        d2col = np.concatenate([d2colE, d2colS])
    else:
        d2col = d2colE
    term2 = np.sqrt(np.maximum(d2col, 0.0)).sum()

    # ---- term1: row mins ----
    term1 = 0.0
    for r in results:
        rsf = np.asarray(r["rowsumE"]).astype(np.float64)  # [P, N_IT*A_G+1]
        rs = rsf[:, : N_IT * A_G].reshape(P, N_IT, A_G)
        softsum = rs.sum(axis=2)  # [P, N_IT]
        softsum[:, 0] += rsf[:, N_IT * A_G]  # tile0 group0 was split in two
        d2soft = -np.log(np.maximum(softsum, 1e-300)) / BETA
        if V_G > 0:
            rm = np.asarray(r["rowmaxS"]).astype(np.float64).reshape(P, N_IT, V_G)
            d2raw = (SHIFT - rm).min(axis=2)  # [P, N_IT]
            d2row = np.minimum(d2soft, d2raw)
        else:
            d2row = d2soft
        term1 += np.sqrt(np.maximum(d2row, 0.0)).sum()

    return np.float32(0.5 * (term1 + term2))


_NC_CACHE = None


def _get_nc():
    global _NC_CACHE
    if _NC_CACHE is None:
        _NC_CACHE = build_nc()
    return _NC_CACHE


def run(set1, set2, trace=False, **trace_kwargs):
    from concourse.bass_utils import run_bass_kernel_spmd

    nc = _get_nc()
    in_maps = make_in_maps(set1, set2)
    res = run_bass_kernel_spmd(
        nc, in_maps, core_ids=list(range(NCORES)), trace=trace, **trace_kwargs
    )
    return combine(res.results), res


def kernel(set1: np.ndarray, set2: np.ndarray) -> np.ndarray:
    out, _ = run(set1, set2, trace=False)
    return np.asarray(out, dtype=np.float32)
